# revision 1
# baseline (speedup 1.0000x reference)
"""KWTA mask kernel for Trainium2, 8-core SPMD.

Algorithm: the mask is (x >= v_K) where v_K is the K-th largest of the
flattened input. v_K is found by distributed bisection on the value axis:

  1. Seed window [-64, 64): brackets v_K for any input with
     |v_K| < 64 (no distribution assumptions; the reference input is
     standard normal so |x| < 6).
  2. 16 quartering rounds: count elements >= 3 interior thresholds of the
     current window (DVE compare+accumulate sweeps), total the counts
     across partitions with a ones-matmul and across cores with an
     add-AllReduce, shrink the window. 16 rounds shrink the 128-wide seed
     window by 4^16 to 2.98e-8, at most 1 fp32 ulp of any v_K with
     |v_K| >= 0.25, at which point lo equals v_K exactly
     (count(>=lo) >= K > count(>=hi) with no representable value strictly
     between lo and hi forces lo == v_K). Counts above 2^24 round in
     fp32, but they are then >> K, so the count>=K decisions are still
     exact.
  3. The final sweep writes the mask BITPACKED (8 mask bits per byte,
     LSB = lowest flat index) so the device->host transfer is 4.2 MB
     instead of 134 MB; the host unpacks with np.unpackbits.

The wall-clock cost is dominated by the host->device transfer of x
(~134 MB); the bisection rounds and collectives are microseconds-scale
on-device and do not register in dispatch wall time.
"""
import numpy as np
import concourse.bass as bass
import concourse.mybir as mybir
from concourse import bass_utils
from concourse.bacc import Bacc
from concourse.tile import TileContext

N_CORES = 8
P = 128
FREE = 32768  # 4,194,304 elements per core / 128 partitions
K = 100000
ROUNDS = 16
SEED_LO = -64.0
SEED_HI = 64.0
ALU = mybir.AluOpType

_cache = {}


def _build():
    dt = mybir.dt
    nc = Bacc(None, target_bir_lowering=False, debug=False)
    x = nc.dram_tensor("x", [P, FREE], dt.float32, kind="ExternalInput")
    y = nc.dram_tensor("y", [P, FREE // 8], dt.uint8, kind="ExternalOutput")
    ccin = nc.dram_tensor("ccin", [P, 3], dt.float32)
    ccout = nc.dram_tensor("ccout", [P, 3], dt.float32, addr_space="Shared")

    with TileContext(nc) as tc:
        with (
            tc.tile_pool(name="big", bufs=1) as big,
            tc.tile_pool(name="small", bufs=1) as small,
            tc.tile_pool(name="mw", bufs=2) as mwp,
            tc.tile_pool(name="mout", bufs=2) as mout,
            tc.tile_pool(name="ps", bufs=1, space="PSUM") as psp,
        ):
            X = big.tile([P, FREE], dt.float32)
            nc.sync.dma_start(out=X[:, :], in_=x[:, :])
            dummy = big.tile([P, FREE], dt.uint8)

            ones = small.tile([P, P], dt.float32)
            nc.vector.memset(ones[:, :], 1.0)
            qc = small.tile([P, 3], dt.float32)
            for j, v in enumerate((0.25, 0.5, 0.75)):
                nc.vector.memset(qc[:, j : j + 1], v)
            # bit weights [P, 1, 8] = 1,2,4,...,128 for packing
            w8 = small.tile([P, 1, 8], dt.float32)
            for k in range(8):
                nc.vector.memset(w8[:, :, k : k + 1], float(1 << k))

            lo = small.tile([P, 1], dt.float32)
            nc.vector.memset(lo[:, :], SEED_LO)
            hi = small.tile([P, 1], dt.float32)
            nc.vector.memset(hi[:, :], SEED_HI)

            # ---- bisection rounds ----------------------------------------
            t3 = small.tile([P, 3], dt.float32)
            cnts = small.tile([P, 3], dt.float32)
            d = small.tile([P, 1], dt.float32)
            ft4 = small.tile([P, 4], dt.float32)
            th4 = small.tile([P, 4], dt.float32)
            gb = small.tile([P, 3], dt.float32)
            f3 = small.tile([P, 3], dt.float32)
            cnt_sb = small.tile([P, 3], dt.float32)

            for _ in range(ROUNDS):
                # t3 = lo + qc * (hi - lo)
                nc.vector.scalar_tensor_tensor(
                    out=d[:, :], in0=hi[:, :], scalar=1.0, in1=lo[:, :],
                    op0=ALU.mult, op1=ALU.subtract,
                )
                nc.vector.scalar_tensor_tensor(
                    out=t3[:, :], in0=qc[:, :], scalar=d[:, :],
                    in1=lo[:, :].broadcast_to([P, 3]),
                    op0=ALU.mult, op1=ALU.add,
                )
                # per-partition counts of (x >= t_j)
                for j in range(3):
                    nc.vector.tensor_scalar(
                        out=dummy[:, :], in0=X[:, :],
                        scalar1=t3[:, j : j + 1], scalar2=0.0,
                        op0=ALU.is_ge, op1=ALU.add,
                        accum_out=cnts[:, j : j + 1],
                    )
                # total across partitions, replicated to every partition
                psum = psp.tile([P, 3], dt.float32)
                nc.tensor.matmul(psum[:, :], ones[:, :], cnts[:, :],
                                 start=True, stop=True)
                nc.vector.tensor_copy(cnt_sb[:, :], psum[:, :])
                nc.sync.dma_start(out=ccin[:, :], in_=cnt_sb[:, :])
                nc.gpsimd.collective_compute(
                    "AllReduce", ALU.add,
                    replica_groups=[list(range(N_CORES))],
                    ins=[ccin[:, :]], outs=[ccout[:, :]],
                )
                nc.sync.dma_start(out=gb[:, :], in_=ccout[:, :])
                # f_j = 1 if global_count_j >= K else 0
                nc.vector.tensor_scalar(
                    out=f3[:, :], in0=gb[:, :], scalar1=float(K), scalar2=None,
                    op0=ALU.is_ge,
                )
                # lo = max(lo, f_j * t_j)
                nc.vector.tensor_copy(ft4[:, 0:1], lo[:, :])
                nc.vector.scalar_tensor_tensor(
                    out=ft4[:, 1:4], in0=f3[:, :], scalar=1.0, in1=t3[:, :],
                    op0=ALU.mult, op1=ALU.mult,
                )
                nc.vector.tensor_reduce(
                    out=lo[:, :], in_=ft4[:, :], axis=mybir.AxisListType.X,
                    op=ALU.max,
                )
                # hi = min(hi, t_j + f_j * BIG)
                nc.vector.tensor_copy(th4[:, 0:1], hi[:, :])
                nc.vector.scalar_tensor_tensor(
                    out=th4[:, 1:4], in0=f3[:, :], scalar=1e30, in1=t3[:, :],
                    op0=ALU.mult, op1=ALU.add,
                )
                nc.vector.tensor_reduce(
                    out=hi[:, :], in_=th4[:, :], axis=mybir.AxisListType.X,
                    op=ALU.min,
                )

            # mask = (x >= lo), bitpacked 8 elements -> 1 byte, streamed out
            NCH = 4
            CH = FREE // NCH      # input elems per chunk
            CHO = CH // 8         # packed bytes per chunk
            for i in range(NCH):
                s = slice(i * CH, (i + 1) * CH)
                so = slice(i * CHO, (i + 1) * CHO)
                mw = mwp.tile([P, CHO, 8], dt.uint8)
                nc.vector.scalar_tensor_tensor(
                    out=mw[:, :, :],
                    in0=X[:, s].rearrange("p (n k) -> p n k", k=8),
                    scalar=lo[:, :],
                    in1=w8[:, :, :].broadcast_to([P, CHO, 8]),
                    op0=ALU.is_ge, op1=ALU.mult,
                )
                pk = mout.tile([P, CHO], dt.uint8)
                with nc.allow_low_precision("bitpack byte sum <= 255, exact"):
                    nc.vector.tensor_reduce(
                        out=pk[:, :], in_=mw[:, :, :],
                        axis=mybir.AxisListType.X, op=ALU.add,
                    )
                nc.sync.dma_start(out=y[:, so], in_=pk[:, :])
    nc.compile()
    return nc


def kernel(x: np.ndarray) -> np.ndarray:
    x = np.asarray(x)
    orig_shape, orig_dtype = x.shape, x.dtype
    flat = np.ascontiguousarray(x, dtype=np.float32).reshape(-1)
    shards = flat.reshape(N_CORES, P, FREE)
    if "nc" not in _cache:
        _cache["nc"] = _build()
    res = bass_utils.run_bass_kernel_spmd(
        _cache["nc"],
        in_maps=[{"x": shards[i]} for i in range(N_CORES)],
        core_ids=list(range(N_CORES)),
    )
    packed = np.concatenate(
        [res.results[i]["y"].reshape(-1) for i in range(N_CORES)]
    )
    out = np.unpackbits(packed, bitorder="little")
    return out.reshape(orig_shape).astype(orig_dtype, copy=False)

